# revision 1
# baseline (speedup 1.0000x reference)
"""Trainium2 Bass kernel for the CRW palindrome-walk contrastive loss.

Reference computation (per batch b):
  f = L2-normalize(feats, axis=C)
  A_t = f_t^T f_{t+1}                      [N,N], t = 0..T-2
  R_t = rowsoftmax(A_t / tau)              (right edges)
  L_t = rowsoftmax(A_t^T / tau)            (left edges)
  for i in 1..T-3:
    path_i = R_0 R_1 .. R_i L_i L_{i-1} .. L_0
    loss_i = -mean_n log_softmax(log(path_i + EPS))[n, n]
  loss = mean_i loss_i

Device algorithm (per core, B/8 = 2 batches):
  * Track Q_i = (R_0..R_i)^T and S_i = L_i..L_0.  Both recurrences use
    naturally-stored operands with the PE's lhsT convention:
       Q_i = matmul(lhsT=R_i,    rhs=Q_{i-1})   (= R_i^T @ Q_{i-1})
       S_i = matmul(lhsT=L_i^T,  rhs=S_{i-1})   (= L_i   @ S_{i-1})
  * R_t  = rowsoftmax(E_t)  with E_t = exp(A_t/tau)      [row scale]
  * L_t^T = colsoftmax(E_t)                               [col scale]
    (colsums via ones-matrix matmul, which also broadcasts across partitions)
  * t=0 is symmetric with E'_0 = exp(A_0^T/tau):
       S_0 = L_0 = rowsoftmax(E'_0),  Q_0 = R_0^T = colsoftmax(E'_0)
  * diag(path_i) = colsum_k(Q_i * S_i) -- no transposes anywhere.
  * Rows of path_i sum to exactly 1 (product of stochastic matrices), so
    log_softmax(log(path+EPS)) diag == log(diag + EPS) - log1p(N*EPS);
    the constant is ~1e-17 and is dropped.
  * Each core returns the [1, N] vector of summed log-diagonals over its
    (i, b); the host sums across cores in float64 and scales.
"""

import threading

import numpy as np

import concourse.bass as bass  # noqa: F401  (engine types come via nc)
import concourse.tile as tile
import concourse.mybir as mybir
from concourse import bacc
from concourse.bass_utils import run_bass_kernel_spmd

B, C, T, N = 16, 128, 8, 1024
NCORES = 8
BPC = B // NCORES          # batches per core
TEMP = 0.07
EPS = 1e-20

F32 = mybir.dt.float32
F32R = mybir.dt.float32r
EXP = mybir.ActivationFunctionType.Exp
LN = mybir.ActivationFunctionType.Ln


def _r(ap):
    """View an fp32 AP as float32r for full-rate PE matmuls."""
    return ap.bitcast(F32R)


def build(n=N, t_len=T, bpc=BPC, n_cores=NCORES, passes=1):
    """Build + compile the per-core Bass program.  Parameterized so tests
    can build a shrunken version for CoreSim.  passes>1 repeats the whole
    computation (timing instrumentation only — output is scaled)."""
    NB = n // 128            # partition blocks per matrix dim
    CHN = 512 if n >= 512 else n
    NCH = n // CHN           # 512-wide chunks per matrix dim
    n_steps = t_len - 2      # walk steps i = 1..n_steps

    nc = bacc.Bacc("TRN2", target_bir_lowering=False, debug=False,
                   num_devices=n_cores)
    # Register EPS as a const AP so `activation(..., bias=EPS)` can use it.
    eps_t = nc.alloc_sbuf_tensor("const-eps", [128, 1], F32)
    nc.gpsimd.memset(eps_t.ap(), EPS)
    nc.const_aps.aps[(F32, EPS)] = eps_t.ap()
    nc.all_engine_barrier()
    feats_d = nc.dram_tensor("feats", [bpc, C, t_len, n], F32,
                             kind="ExternalInput")
    out_d = nc.dram_tensor("out", [1, n], F32, kind="ExternalOutput")
    feats_ap = feats_d.ap()
    out_ap = out_d.ap()

    with tile.TileContext(nc) as tc:
        with (
            # SBUF pools (per-partition bytes in comments)
            tc.tile_pool(name="const", bufs=1) as const_pool,            # ones: 512B
            tc.tile_pool(name="slice", bufs=1) as slice_pool,    # raw f: 1x4K
            tc.tile_pool(name="fh", bufs=2) as fh_pool,          # fhat: 2x4K
            tc.tile_pool(name="e", bufs=2 * NB) as e_pool,       # 16x4K
            tc.tile_pool(name="q", bufs=3 * NB) as q_pool,       # 24x2K
            tc.tile_pool(name="s", bufs=3 * NB) as s_pool,       # 24x2K
            tc.tile_pool(name="d", bufs=2) as d_pool,            # 2x2K
            tc.tile_pool(name="cs", bufs=2) as cs_pool,          # nrm 2x2K
            tc.tile_pool(name="stat", bufs=2 * NB) as stat_pool, # [128,1]s
            tc.tile_pool(name="lg", bufs=2) as lg_pool,          # [1,CHN]
            tc.tile_pool(name="acc", bufs=1) as acc_pool,                # [1,n]
            # PSUM pools -- 8 banks total
            tc.tile_pool(name="aps", bufs=2, space="PSUM") as aps,    # 4 banks
            tc.tile_pool(name="qps", bufs=1, space="PSUM") as qps,    # 1
            tc.tile_pool(name="sps", bufs=1, space="PSUM") as sps,    # 1
            tc.tile_pool(name="csps", bufs=2, space="PSUM") as csps,  # 2
        ):
            ones_raw = const_pool.tile([128, 128], F32, tag="ones_raw")
            nc.vector.memset(ones_raw[:], 1.0)
            ones = const_pool.tile([128, 128], F32R, tag="ones")
            nc.scalar.copy(ones[:], ones_raw[:])
            loss_acc = acc_pool.tile([1, n], F32, tag="acc")
            nc.vector.memset(loss_acc[:], 0.0)

            def chs(ch):
                return slice(ch * CHN, (ch + 1) * CHN)

            def load_slice(b, t):
                """DMA feats[b,:,t,:] then L2-normalize columns -> fhat."""
                f = slice_pool.tile([128, n], F32, tag="fraw")
                nc.sync.dma_start(f[:], feats_ap[b, :, t, :])
                fh = fh_pool.tile([128, n], F32R, tag="fh")
                for ch in range(NCH):
                    sq = d_pool.tile([128, CHN], F32R, tag="d")
                    nc.scalar.square(sq[:], f[:, chs(ch)])
                    nps = csps.tile([128, CHN], F32, tag="cps")
                    nc.tensor.matmul(nps[:], _r(ones[:]), _r(sq[:]),
                                     start=True, stop=True)
                    nrm = cs_pool.tile([128, CHN], F32, tag="cs")
                    nc.scalar.sqrt(nrm[:], nps[:])
                    nc.vector.tensor_scalar_add(nrm[:], nrm[:], 1e-12)
                    nc.vector.reciprocal(nrm[:], nrm[:])
                    nc.vector.tensor_mul(fh[:, chs(ch)], f[:, chs(ch)], nrm[:])
                return fh

            def softmax_pair(t, fL, fR, want_cr=True):
                """Affinity + exp for one timestep.

                Returns (e_t[NB] E tiles [128,n],
                         rr[NB]  [128,1] 1/rowsum(E)  (row-softmax scale),
                         cr[NB]  [128,1] 1/colsum(E) per output block
                                 (col-softmax scale, partition-wise)).
                The softmax scales are never applied to E; they are folded
                into the walk (rr: rhs row prescale; cr: S-evac scale).
                """
                e_t, rs_t = [], []
                for nb in range(NB):
                    a_ps = aps.tile([128, n], F32, tag="aps")
                    for ch in range(NCH):
                        nc.tensor.matmul(
                            a_ps[:, chs(ch)],
                            fL[:, nb * 128:(nb + 1) * 128],
                            fR[:, chs(ch)],
                            start=True, stop=True)
                    e = e_pool.tile([128, n], F32R, tag="e")
                    rs = stat_pool.tile([128, 1], F32, tag="rs")
                    nc.scalar.activation(e[:], a_ps[:], EXP,
                                         scale=1.0 / TEMP, accum_out=rs[:])
                    e_t.append(e)
                    rs_t.append(rs)

                rr = []
                for nb in range(NB):
                    r = stat_pool.tile([128, 1], F32, tag="rr")
                    nc.vector.reciprocal(r[:], rs_t[nb][:])
                    rr.append(r)

                cr = []
                if want_cr:
                    # c[m] = sum_k E[k, m], as a per-partition vector per
                    # output block: lhsT = E[:, mb-block], rhs = ones column.
                    # (fp32r matmul requires moving free dim >= 2)
                    for mb in range(NB):
                        mbs = slice(mb * 128, (mb + 1) * 128)
                        c_ps = csps.tile([128, 2], F32, tag="cps")
                        for kb in range(NB):
                            nc.tensor.matmul(c_ps[:], e_t[kb][:, mbs],
                                             ones[:, 0:2],
                                             start=(kb == 0),
                                             stop=(kb == NB - 1))
                        c = stat_pool.tile([128, 1], F32, tag="cr")
                        nc.vector.reciprocal(c[:], c_ps[:, 0:1])
                        cr.append(c)
                return e_t, rr, cr

            def t0_setup(f0, f1):
                """Seed both chains from E' = exp(A_0^T / tau).

                S_0 = rowsoftmax(E') materialized; Q_0 = colsoftmax(E') is
                kept RAW (= E' tiles) with its column scale 1/c'[j] folded
                into the loss: acc[j] -= n_steps * log(c'[j])."""
                e_t, rr, _ = softmax_pair(0, fL=f1, fR=f0, want_cr=False)
                s_cur = [[None] * NCH for _ in range(NB)]
                for nb in range(NB):
                    for ch in range(NCH):
                        s = s_pool.tile([128, CHN], F32R, tag="s")
                        nc.vector.tensor_scalar_mul(
                            s[:], e_t[nb][:, chs(ch)], rr[nb][:])
                        s_cur[nb][ch] = s[:]
                for ch in range(NCH):
                    c_ps = csps.tile([1, CHN], F32, tag="cps")
                    for kb in range(NB):
                        nc.tensor.matmul(c_ps[:], ones[:, 0:1],
                                         e_t[kb][:, chs(ch)],
                                         start=(kb == 0), stop=(kb == NB - 1))
                    lgc = lg_pool.tile([1, CHN], F32, tag="lg")
                    nc.scalar.activation(lgc[:], c_ps[:], LN)
                    nc.vector.tensor_scalar(
                        out=lgc[:], in0=lgc[:], scalar1=-float(n_steps),
                        scalar2=None, op0=mybir.AluOpType.mult)
                    nc.vector.tensor_add(loss_acc[:, chs(ch)],
                                         loss_acc[:, chs(ch)], lgc[:])
                q_cur = [[e_t[nb][:, chs(ch)] for ch in range(NCH)]
                         for nb in range(NB)]
                return q_cur, s_cur

            def walk_step(e_t, rr, cr, q_prev, s_prev):
                """One palindrome step: extend both chains, add diag loss.

                Both chain matmuls take lhsT = E_t directly; the row-softmax
                scale rr is pre-applied to Q_{i-1} rows (contraction side),
                and the col-softmax scale cr is applied on the S evacuation
                (output rows)."""
                for kb in range(NB):
                    for ch in range(NCH):
                        nc.vector.tensor_scalar_mul(
                            q_prev[kb][ch], q_prev[kb][ch], rr[kb][:])
                q_new = [[None] * NCH for _ in range(NB)]
                s_new = [[None] * NCH for _ in range(NB)]
                for ch in range(NCH):
                    for mb in range(NB):
                        mbs = slice(mb * 128, (mb + 1) * 128)
                        qp = qps.tile([128, CHN], F32, tag="qps")
                        for kb in range(NB):
                            nc.tensor.matmul(qp[:], e_t[kb][:, mbs],
                                             q_prev[kb][ch],
                                             start=(kb == 0),
                                             stop=(kb == NB - 1))
                        qn = q_pool.tile([128, CHN], F32R, tag="q")
                        nc.vector.tensor_copy(qn[:], qp[:])
                        q_new[mb][ch] = qn[:]

                        sp = sps.tile([128, CHN], F32, tag="sps")
                        for kb in range(NB):
                            nc.tensor.matmul(sp[:], e_t[kb][:, mbs],
                                             s_prev[kb][ch],
                                             start=(kb == 0),
                                             stop=(kb == NB - 1))
                        sn = s_pool.tile([128, CHN], F32R, tag="s")
                        nc.vector.tensor_scalar_mul(sn[:], sp[:], cr[mb][:])
                        s_new[mb][ch] = sn[:]

                # diag(path) = colsum_k (Q * S); log; accumulate
                for ch in range(NCH):
                    d_ps = csps.tile([1, CHN], F32, tag="cps")
                    for kb in range(NB):
                        d = d_pool.tile([128, CHN], F32R, tag="d")
                        nc.vector.tensor_mul(d[:], q_new[kb][ch],
                                             s_new[kb][ch])
                        nc.tensor.matmul(d_ps[:], ones[:, 0:1], d[:],
                                         start=(kb == 0), stop=(kb == NB - 1))
                    lg = lg_pool.tile([1, CHN], F32, tag="lg")
                    nc.scalar.activation(lg[:], d_ps[:], LN, bias=EPS)
                    nc.vector.tensor_add(loss_acc[:, chs(ch)],
                                         loss_acc[:, chs(ch)], lg[:])
                return q_new, s_new

            for b in [bb for _ in range(passes) for bb in range(bpc)]:
                f0 = load_slice(b, 0)
                f1 = load_slice(b, 1)
                q_cur, s_cur = t0_setup(f0, f1)
                f_prev = f1
                for t in range(1, n_steps + 1):
                    f_next = load_slice(b, t + 1)
                    e_t, rr, cr = softmax_pair(t, fL=f_prev, fR=f_next)
                    q_cur, s_cur = walk_step(e_t, rr, cr, q_cur, s_cur)
                    f_prev = f_next

            nc.sync.dma_start(out_ap[:, :], loss_acc[:])

    nc.compile()
    return nc


_build_lock = threading.Lock()
_built_nc = None


def _get_nc():
    global _built_nc
    with _build_lock:
        if _built_nc is None:
            _built_nc = build()
    return _built_nc


LAST_RESULT = None  # BassKernelResults of the most recent run (for profiling)


def kernel(feats: np.ndarray) -> np.ndarray:
    global LAST_RESULT
    feats = np.ascontiguousarray(np.asarray(feats), dtype=np.float32)
    assert feats.shape == (B, C, T, N), feats.shape
    nc = _get_nc()
    in_maps = [
        {"feats": np.ascontiguousarray(feats[c * BPC:(c + 1) * BPC])}
        for c in range(NCORES)
    ]
    res = run_bass_kernel_spmd(nc, in_maps, core_ids=list(range(NCORES)))
    LAST_RESULT = res
    total = 0.0
    for r in res.results:
        total += r["out"].astype(np.float64).sum()
    n_walks = T - 2  # i = 1..T-2 inclusive
    loss = -total / (n_walks * B * N)
    return np.float32(loss)



# revision 7
# speedup vs baseline: 1.2897x; 1.2897x over previous
"""Trainium2 Bass kernel for the CRW palindrome-walk contrastive loss.

Reference computation (per batch b):
  f = L2-normalize(feats, axis=C)
  A_t = f_t^T f_{t+1}                      [N,N], t = 0..T-2
  R_t = rowsoftmax(A_t / tau)              (right edges)
  L_t = rowsoftmax(A_t^T / tau)            (left edges)
  for i in 1..T-3:
    path_i = R_0 R_1 .. R_i L_i L_{i-1} .. L_0
    loss_i = -mean_n log_softmax(log(path_i + EPS))[n, n]
  loss = mean_i loss_i

Device algorithm (per core, B/8 = 2 batches), fp8 DoubleRow edition:
  * Track Q_i = (R_0..R_i)^T and S_i = L_i..L_0 as fp8e4m3 tensors scaled
    by AL=128 (entries of both are [0,1] col-distributions, so AL*x fits
    e4m3's 240 max with margin; subnormal floor 2^-9/AL = 1.5e-5 of a
    unit column is noise).
  * Per pair t, quantize the two normalized stochastic matrices straight
    from E = exp(A_t/tau):
       Rq[k,m] = AL * E[k,m] / rowsum_k   (Q-chain weights, row-softmax)
       Lq[k,m] = AL * E[k,m] / colsum_m   (S-chain weights, col-softmax^T)
    Both recurrences then need NO per-step rescales:
       qp = Rq^T @ qt = AL^2 Q_i   -> evac qt' = fp8(qp/AL)
       sp = Lq^T @ st = AL^2 S_i   -> evac st' = fp8(sp/AL)
    Weights/moving both fp8 -> MatmulPerfMode.DoubleRow contracts 256
    rows per pass (tiles laid out [128, 2, n], dim1 = k-block pair).
  * t=0 is the same pair_quant with E' = exp(A_0^T/tau): its row-softmax
    IS S_0 = L_0 and its col-softmax IS Q_0 = R_0^T.
  * diag(path_i) = colsum_k (Q*S): d = (qt*(1/AL))*st in bf16 (= AL*Q*S),
    ones-matmul colsum, ln, accumulate; host subtracts ln(AL).
  * Rows of path_i sum to 1, so log_softmax == log(diag + EPS).
  * Each core returns the [1, N] vector of summed log-diagonals over its
    (i, b); the host sums across cores in float64 and corrects scales.
"""

import threading

import numpy as np

import concourse.bass as bass  # noqa: F401  (engine types come via nc)
import concourse.tile as tile
import concourse.mybir as mybir
from concourse import bacc
from concourse.bass_utils import run_bass_kernel_spmd

B, C, T, N = 16, 128, 8, 1024
NCORES = 8
BPC = B // NCORES          # batches per core
TEMP = 0.07
EPS = 1e-20
AL = 128.0                 # fp8 storage scale for Q/S/Rq/Lq

F32 = mybir.dt.float32
F32R = mybir.dt.float32r
BF16 = mybir.dt.bfloat16
FP8 = mybir.dt.float8e4
EXP = mybir.ActivationFunctionType.Exp
LN = mybir.ActivationFunctionType.Ln
COPY = mybir.ActivationFunctionType.Copy
MUL = mybir.AluOpType.mult
DR = mybir.MatmulPerfMode.DoubleRow


def _r(ap):
    """View an fp32 AP as float32r for full-rate PE matmuls."""
    return ap.bitcast(F32R)


def build(n=N, t_len=T, bpc=BPC, n_cores=NCORES, passes=1):
    """Build + compile the per-core Bass program.  Parameterized so tests
    can build a shrunken version for CoreSim.  passes>1 repeats the whole
    computation (timing instrumentation only — output is scaled)."""
    NB = n // 128            # partition blocks per matrix dim
    KP = NB // 2             # k-block pairs (DoubleRow granules)
    CHN = 512 if n >= 512 else n
    NCH = n // CHN           # psum-bank-wide chunks per matrix dim
    n_steps = t_len - 2      # walk steps i = 1..n_steps

    nc = bacc.Bacc("TRN2", target_bir_lowering=False, debug=False,
                   num_devices=n_cores)
    # Register EPS as a const AP so `activation(..., bias=EPS)` can use it.
    eps_t = nc.alloc_sbuf_tensor("const-eps", [128, 1], F32)
    nc.gpsimd.memset(eps_t.ap(), EPS)
    nc.const_aps.aps[(F32, EPS)] = eps_t.ap()
    nc.all_engine_barrier()
    feats_d = nc.dram_tensor("feats", [bpc, C, t_len, n], F32,
                             kind="ExternalInput")
    out_d = nc.dram_tensor("out", [1, n], F32, kind="ExternalOutput")
    feats_ap = feats_d.ap()
    out_ap = out_d.ap()

    with tile.TileContext(nc) as tc:
        with (
            # SBUF pools (per-partition bytes in comments)
            tc.tile_pool(name="const", bufs=1) as const_pool,    # ~1.3KB
            tc.tile_pool(name="slice", bufs=2) as slice_pool,    # raw f 2x4K
            tc.tile_pool(name="fh", bufs=3) as fh_pool,          # fhat 3x4K
            tc.tile_pool(name="e", bufs=2 * NB) as e_pool,       # bf16 16x2K
            tc.tile_pool(name="rl", bufs=2 * KP) as rl_pool,     # fp8 2tag x8x2K
            tc.tile_pool(name="q", bufs=2 * KP) as q_pool,       # fp8 8x2K
            tc.tile_pool(name="s", bufs=2 * KP) as s_pool,       # fp8 8x2K
            tc.tile_pool(name="d", bufs=2 * NB + 2) as d_pool,   # bf16 18x1K
            tc.tile_pool(name="crb", bufs=2) as crb_pool,        # bf16 2x1K
            tc.tile_pool(name="cs", bufs=2) as cs_pool,          # nrm 2x2K
            tc.tile_pool(name="stat", bufs=4 * NB) as stat_pool, # [128,1]s
            tc.tile_pool(name="crow", bufs=2) as crow_pool,      # [1,n]
            tc.tile_pool(name="lg", bufs=2) as lg_pool,          # [1,CHN]
            tc.tile_pool(name="acc", bufs=1) as acc_pool,        # [1,n]
            # PSUM pools -- 8 banks total
            tc.tile_pool(name="aps", bufs=2, space="PSUM") as aps,    # 4 banks
            tc.tile_pool(name="qsps", bufs=3, space="PSUM") as qsps,  # 3
            tc.tile_pool(name="csps", bufs=1, space="PSUM") as csps,  # 1
        ):
            ones_raw = const_pool.tile([128, 128], F32, tag="ones_raw")
            nc.vector.memset(ones_raw[:], 1.0)
            ones = const_pool.tile([128, 128], F32R, tag="ones")
            nc.scalar.copy(ones[:], ones_raw[:])
            ones_bf = const_pool.tile([128, 2], BF16, tag="ones_bf")
            nc.vector.memset(ones_bf[:], 1.0)
            # [1,128] row of AL for the colsum-reciprocal broadcast matmul
            alrow = const_pool.tile([1, 128], BF16, tag="alrow")
            nc.vector.memset(alrow[:], AL)
            loss_acc = acc_pool.tile([1, n], F32, tag="acc")
            nc.vector.memset(loss_acc[:], 0.0)

            def chs(ch):
                return slice(ch * CHN, (ch + 1) * CHN)

            def load_slice(b, t):
                """DMA feats[b,:,t,:] then L2-normalize columns -> fhat."""
                f = slice_pool.tile([128, n], F32, tag="fraw")
                nc.sync.dma_start(f[:], feats_ap[b, :, t, :])
                fh = fh_pool.tile([128, n], F32R, tag="fh")
                for ch in range(NCH):
                    sq = d_pool.tile([128, CHN], F32R, tag="dsq")
                    nc.scalar.square(sq[:], f[:, chs(ch)])
                    nps = csps.tile([128, CHN], F32, tag="cps")
                    nc.tensor.matmul(nps[:], _r(ones[:]), _r(sq[:]),
                                     start=True, stop=True)
                    nrm = cs_pool.tile([128, CHN], F32, tag="cs")
                    nc.scalar.sqrt(nrm[:], nps[:])
                    nc.vector.tensor_scalar_add(nrm[:], nrm[:], 1e-12)
                    nc.vector.reciprocal(nrm[:], nrm[:])
                    nc.vector.tensor_mul(fh[:, chs(ch)], f[:, chs(ch)], nrm[:])
                return fh

            def pair_quant(fL, fR, out_r, out_l):
                """Affinity + exp + both softmax quantizations for one pair.

                out_r[kp][:, kb%2, :] <- AL * rowsoftmax(E)[kb-block]
                out_l[kp][:, kb%2, :] <- AL * colsoftmax(E)[kb-block]
                where E = exp(fL^T fR / tau), contraction k on partitions.
                """
                e_t = []
                # affinity + exp (rowsums accumulate on the fly)
                for nb in range(NB):
                    a_ps = aps.tile([128, n], F32, tag="aps")
                    for ch in range(NCH):
                        nc.tensor.matmul(
                            a_ps[:, chs(ch)],
                            fL[:, nb * 128:(nb + 1) * 128],
                            fR[:, chs(ch)],
                            start=True, stop=True)
                    e = e_pool.tile([128, n], BF16, tag="e")
                    rs = stat_pool.tile([128, 1], F32, tag="rs")
                    nc.scalar.activation(e[:], a_ps[:], EXP,
                                         scale=1.0 / TEMP, accum_out=rs[:])
                    e_t.append((e, rs))
                    # row-softmax quant: out = (e * (1/rs)) * AL
                    rrec = stat_pool.tile([128, 1], F32, tag="rrec")
                    nc.vector.reciprocal(rrec[:], rs[:])
                    nc.vector.tensor_scalar(
                        out=out_r[nb // 2][:, nb % 2, :], in0=e[:],
                        scalar1=rrec[:], scalar2=AL, op0=MUL, op1=MUL)
                # colsums -> [1, n] -> bf16 recip -> broadcast AL/cs
                for ch in range(NCH):
                    c_ps = csps.tile([1, CHN], F32, tag="cps")
                    for kb in range(NB):
                        nc.tensor.matmul(c_ps[:], ones_bf[:, 0:1],
                                         e_t[kb][0][:, chs(ch)],
                                         start=(kb == 0), stop=(kb == NB - 1))
                    crec = crow_pool.tile([1, CHN], BF16, tag="crec")
                    with nc.allow_low_precision(reason="bf16 colsum recip"):
                        nc.vector.reciprocal(crec[:], c_ps[:])
                    cb_ps = csps.tile([128, CHN], F32, tag="cps")
                    nc.tensor.matmul(cb_ps[:], alrow[:], crec[:],
                                     start=True, stop=True)
                    crb = crb_pool.tile([128, CHN], BF16, tag="crb")
                    nc.scalar.copy(crb[:], cb_ps[:])
                    # col-softmax quant: out = (e * 1) * (AL/cs)
                    for kb in range(NB):
                        nc.vector.scalar_tensor_tensor(
                            out=out_l[kb // 2][:, kb % 2, chs(ch)],
                            in0=e_t[kb][0][:, chs(ch)], scalar=1.0,
                            in1=crb[:], op0=MUL, op1=MUL)

            def alloc_rl(pool, tag):
                return [pool.tile([128, 2, n], FP8, tag=tag,
                                  name=f"{tag}{kp}")
                        for kp in range(KP)]

            def walk_step(rq, lq, q_cur, s_cur):
                """One palindrome step: extend both chains, add diag loss."""
                q_new = alloc_rl(q_pool, "q")
                s_new = alloc_rl(s_pool, "s")
                for mb in range(NB):
                    mbs = slice(mb * 128, (mb + 1) * 128)
                    for ch in range(NCH):
                        qp = qsps.tile([128, CHN], F32, tag="qsps")
                        for kp in range(KP):
                            nc.tensor.matmul(qp[:], rq[kp][:, :, mbs],
                                             q_cur[kp][:, :, chs(ch)],
                                             start=(kp == 0),
                                             stop=(kp == KP - 1),
                                             perf_mode=DR)
                        nc.scalar.activation(
                            q_new[mb // 2][:, mb % 2, chs(ch)], qp[:],
                            COPY, scale=1.0 / AL)
                        sp = qsps.tile([128, CHN], F32, tag="qsps")
                        for kp in range(KP):
                            nc.tensor.matmul(sp[:], lq[kp][:, :, mbs],
                                             s_cur[kp][:, :, chs(ch)],
                                             start=(kp == 0),
                                             stop=(kp == KP - 1),
                                             perf_mode=DR)
                        nc.vector.tensor_scalar_mul(
                            s_new[mb // 2][:, mb % 2, chs(ch)], sp[:],
                            1.0 / AL)
                        # d = (qt/AL) * st = AL * Q * S, ready for colsum
                        d = d_pool.tile([128, CHN], BF16, tag="d")
                        nc.vector.scalar_tensor_tensor(
                            out=d[:], in0=q_new[mb // 2][:, mb % 2, chs(ch)],
                            scalar=1.0 / AL,
                            in1=s_new[mb // 2][:, mb % 2, chs(ch)],
                            op0=MUL, op1=MUL)
                        walk_step.d_tiles[(mb, ch)] = d
                # diag(path) = colsum_k d; log; accumulate
                for ch in range(NCH):
                    d_ps = csps.tile([1, CHN], F32, tag="cps")
                    for kb in range(NB):
                        nc.tensor.matmul(d_ps[:], ones_bf[:, 0:1],
                                         walk_step.d_tiles[(kb, ch)][:],
                                         start=(kb == 0), stop=(kb == NB - 1))
                    lg = lg_pool.tile([1, CHN], F32, tag="lg")
                    nc.scalar.activation(lg[:], d_ps[:], LN, bias=EPS)
                    nc.vector.tensor_add(loss_acc[:, chs(ch)],
                                         loss_acc[:, chs(ch)], lg[:])
                return q_new, s_new

            walk_step.d_tiles = {}

            for b in [bb for _ in range(passes) for bb in range(bpc)]:
                f0 = load_slice(b, 0)
                f1 = load_slice(b, 1)
                # t0: row-softmax of E' is S_0, col-softmax of E' is Q_0
                s_cur = alloc_rl(s_pool, "s")
                q_cur = alloc_rl(q_pool, "q")
                pair_quant(fL=f1, fR=f0, out_r=s_cur, out_l=q_cur)
                f_prev = f1
                rq = alloc_rl(rl_pool, "r")
                lq = alloc_rl(rl_pool, "l")
                f_next = load_slice(b, 2)
                pair_quant(fL=f_prev, fR=f_next, out_r=rq, out_l=lq)
                f_prev = f_next
                for t in range(1, n_steps + 1):
                    q_cur, s_cur = walk_step(rq, lq, q_cur, s_cur)
                    if t < n_steps:
                        rq = alloc_rl(rl_pool, "r")
                        lq = alloc_rl(rl_pool, "l")
                        f_next = load_slice(b, t + 2)
                        pair_quant(fL=f_prev, fR=f_next, out_r=rq, out_l=lq)
                        f_prev = f_next

            nc.sync.dma_start(out_ap[:, :], loss_acc[:])

    nc.compile()
    return nc


_build_lock = threading.Lock()
_built_nc = None


def _get_nc():
    global _built_nc
    with _build_lock:
        if _built_nc is None:
            _built_nc = build()
    return _built_nc


LAST_RESULT = None  # BassKernelResults of the most recent run (for profiling)


def kernel(feats: np.ndarray) -> np.ndarray:
    global LAST_RESULT
    feats = np.ascontiguousarray(np.asarray(feats), dtype=np.float32)
    assert feats.shape == (B, C, T, N), feats.shape
    nc = _get_nc()
    in_maps = [
        {"feats": np.ascontiguousarray(feats[c * BPC:(c + 1) * BPC])}
        for c in range(NCORES)
    ]
    res = run_bass_kernel_spmd(nc, in_maps, core_ids=list(range(NCORES)))
    LAST_RESULT = res
    total = 0.0
    for r in res.results:
        total += r["out"].astype(np.float64).sum()
    n_walks = T - 2  # i = 1..T-2 inclusive
    loss = -(total / (n_walks * B * N) - np.log(AL))
    return np.float32(loss)
